# revision 1
# baseline (speedup 1.0000x reference)
"""Causal self-attention (B=2, T=2048, C=1024, H=16) on 8 trn2 NeuronCores.

Sharding: core = (batch b, head-group g) with 4 heads per group.
  - data parallel over B (2 ways) x tensor parallel over heads (4 ways)
  - each core computes qkv for its head group, causal attention for its
    4 heads, and a partial proj (its 256 rows of w_proj); the host sums
    the 4 per-batch partials (deferred tensor-parallel all-reduce).

Device layout (per core, all fp32 in memory, float32r on the PE;
inputs host-prearranged chunk-major so every DMA is contiguous):
  xt    [8, 4, 128, 512]  x^T chunks (c-chunk, t-block, partition, t)
  wqkv  [8, 128, 768]     [q_g | k_g | v_g] chunks for this group
  wproj [2, 128, 1024]    w_proj row-chunks for this group
  masks [128, 2048]   4 causal staircase masks (diag offsets 0..3 * 128)
  out   [2048, 1024]  partial output (host sums groups)

The attention math is arranged so no on-device transposes are needed:
  q^T,k^T [d, t] come straight out of the qkv matmul (lhsT = w slice,
  rhs = x^T); S^T[tk,tq] = k^T.T @ q^T-moving; exp on ACT; y^T and the
  softmax denominator come from one AV matmul with a ones-column
  appended to v (lhsT = v_aug), and proj consumes y^T directly as lhsT.
Softmax skips max-subtraction (scores ~ N(0,1) after 1/sqrt(D): exp is
safe in fp32), matching the reference up to fp rounding.
"""

import os
import sys
from contextlib import ExitStack

import numpy as np

for _p in ("/opt/trn_rl_repo", "/root/.axon_site/_ro/trn_rl_repo"):
    if os.path.isdir(_p) and _p not in sys.path:
        sys.path.insert(0, _p)

import concourse.bass as bass
import concourse.bacc as bacc
import concourse.mybir as mybir
import concourse.tile as tile
from concourse.bass_utils import run_bass_kernel_spmd

B, T, C, H, D = 2, 2048, 1024, 16, 64
GH = 4                 # heads per core (group)
GC = GH * D            # 256 channels per group
NCORES = 8
TQ = 512               # query tile (free dim of S^T / AV matmuls)
TK = 128               # key tile (partition dim of S^T)
NB = T // TQ           # 4 query blocks
NT = T // TK           # 16 key tiles
CK = C // 128          # 8 contraction chunks for qkv
F32 = mybir.dt.float32
F32R = mybir.dt.float32r

EXPF = mybir.ActivationFunctionType.Exp

_CACHE = {}


def _r(ap):
    """View an fp32 AP as float32r (TF32; same 4-byte container)."""
    return ap.bitcast(mybir.dt.float32r)


def _build_bass(repeat=1):
    nc = bacc.Bacc("TRN2", target_bir_lowering=False, debug=False)
    xt = nc.declare_dram_parameter("xt", [CK, NB, 128, TQ], F32R, isOutput=False)
    wqkv = nc.declare_dram_parameter("wqkv", [CK, 128, 3 * GC], F32R, isOutput=False)
    wproj = nc.declare_dram_parameter("wproj", [2, 128, C], F32R, isOutput=False)
    masks = nc.declare_dram_parameter("masks", [128, 4 * TQ], F32R, isOutput=False)
    out = nc.declare_dram_parameter("out", [T, C], F32, isOutput=True)

    with ExitStack() as ctx:
        tc = ctx.enter_context(tile.TileContext(nc))
        consts = ctx.enter_context(tc.tile_pool(name="consts", bufs=1))
        persist = ctx.enter_context(tc.tile_pool(name="persist", bufs=1))
        xpool = ctx.enter_context(tc.tile_pool(name="xp", bufs=2))
        espool = ctx.enter_context(tc.tile_pool(name="es", bufs=4))
        rpool = ctx.enter_context(tc.tile_pool(name="rp", bufs=2))
        rbpool = ctx.enter_context(tc.tile_pool(name="rb", bufs=2))
        opool = ctx.enter_context(tc.tile_pool(name="op", bufs=3))
        dpool = ctx.enter_context(tc.tile_pool(name="dp", bufs=2, space="DRAM"))
        # 8 fixed PSUM tiles (one bank each), rotated manually. Pool-based
        # PSUM slot reuse emits unconditional multi-sem release waits on the
        # claiming matmul, which blows the 1-wait budget of f32r self-loading
        # matmuls; with fixed tiles reuse is plain WAR/WAW dep tracking.
        psum = ctx.enter_context(tc.tile_pool(name="psum", bufs=1, space="PSUM"))
        PSD = [
            psum.tile([128, 2, TQ], F32, tag=f"psd{r}", name=f"psd{r}")
            for r in range(2)
        ]
        PP = [psum.tile([128, TQ], F32, tag="pp0", name="pp0")]
        PJ = [psum.tile([128, TQ], F32, tag="pj0", name="pj0")]
        AVD = psum.tile([D + 1, 2, TQ], F32, tag="avd", name="avd")
        cnt = {"pp": 0, "pj": 0, "ps": 0}

        # ---- constants / persistent tiles ----
        # (DMAs for weights/masks are issued inside qkv_block(0), interleaved
        # after the x chunks, so the first matmuls start as early as possible)
        w_sb = consts.tile([128, CK, 3 * GC], F32R, tag="wqkv")
        wp_sb = consts.tile([128, 2, C], F32R, tag="wproj")
        mk_sb = consts.tile([128, 4 * TQ], F32R, tag="masks")

        qT = [
            persist.tile([128, T], F32R, tag=f"qT{p}", name=f"qT{p}") for p in range(2)
        ]
        kT = [
            persist.tile([128, T], F32R, tag=f"kT{p}", name=f"kT{p}") for p in range(2)
        ]
        yT = [
            persist.tile([128, T], F32R, tag=f"yT{p}", name=f"yT{p}") for p in range(2)
        ]
        vaug = persist.tile([128, NT, GH * (D + 1)], F32R, tag="vaug")

        def load_consts_tail():
            nc.sync.dma_start(out=mk_sb[:], in_=masks[:])
            for k in range(2):
                nc.sync.dma_start(out=wp_sb[:, k, :], in_=wproj[k, :, :])
            for h in range(GH):
                # mk_sb[:, TQ-1] is all-ones (d=0 staircase, last column)
                nc.vector.tensor_copy(
                    vaug[:, :, h * (D + 1) + D : (h + 1) * (D + 1)],
                    mk_sb[:, TQ - 1 : TQ].unsqueeze(1).broadcast_to((128, NT, 1)),
                )

        def qkv_block(tb, rep):
            x_sb = xpool.tile([128, CK, TQ], F32R, tag="x")
            first = tb == 0 and rep == 0
            for k in range(CK):
                nc.sync.dma_start(out=x_sb[:, k, :], in_=xt[k, tb, :, :])
                if first:
                    # q/k weight columns first - they gate the first matmuls;
                    # v columns follow after all x/qk chunks are queued
                    nc.sync.dma_start(
                        out=w_sb[:, k, 0 : 2 * GC], in_=wqkv[k, :, 0 : 2 * GC]
                    )
            if first:
                for k in range(CK):
                    nc.sync.dma_start(
                        out=w_sb[:, k, 2 * GC : 3 * GC],
                        in_=wqkv[k, :, 2 * GC : 3 * GC],
                    )
                load_consts_tail()
            # q^T / k^T for both head pairs. In block 0 the proj bank PJ is
            # provably idle (no yT yet), so ping-pong groups across PP/PJ to
            # hide the psum->sbuf copy latency on the startup critical path.
            for pair in range(2):
                for which, dest in ((0, qT), (1, kT)):
                    pq = (PP + PJ)[cnt["pp"] % 2] if first else PP[0]
                    cnt["pp"] += 1
                    for k in range(CK):
                        cols = which * GC + pair * 128
                        nc.tensor.matmul(
                            pq[:],
                            w_sb[:, k, cols : cols + 128],
                            x_sb[:, k, :],
                            start=(k == 0),
                            stop=(k == CK - 1),
                        )
                    nc.vector.tensor_copy(
                        dest[pair][:, tb * TQ : (tb + 1) * TQ], _r(pq[:])
                    )
            # v for the 4 key tiles of this block
            for tt in range(TQ // TK):
                jt = tb * (TQ // TK) + tt
                pv = (PP + PJ)[cnt["pp"] % 2] if first else PP[0]
                cnt["pp"] += 1
                for k in range(CK):
                    nc.tensor.matmul(
                        pv[:, 0:GC],
                        x_sb[:, k, tt * TK : (tt + 1) * TK],
                        w_sb[:, k, 2 * GC : 3 * GC],
                        start=(k == 0),
                        stop=(k == CK - 1),
                    )
                nc.vector.tensor_copy(
                    vaug[:, jt, :]
                    .rearrange("p (h c) -> p h c", h=GH)[:, :, 0:D],
                    _r(pv[:, 0:GC].rearrange("p (h c) -> p h c", h=GH)),
                )

        def attn_block(pair, i, last=False):
            jmax = (TQ // TK) * (i + 1)
            for j in range(jmax):
                dg = j - (TQ // TK) * i  # >=0 on the diagonal band
                # valid region of this tile is cols [dg*TK, TQ); compute only
                # [c0, TQ) with c0 capped so the f32r moving dim stays >=256
                c0 = min(dg, 2) * TK if dg > 0 else 0
                cs = slice(c0, TQ)
                psd = PSD[cnt["ps"] % 2]
                cnt["ps"] += 1
                for half in range(2):
                    lo, hi = half * D, half * D + D
                    kap = kT[pair][lo:hi, j * TK : (j + 1) * TK]
                    qap = qT[pair][lo:hi, i * TQ + c0 : (i + 1) * TQ]
                    nc.tensor.matmul(
                        psd[:, half, cs], kap, qap, start=True, stop=True
                    )
                est = espool.tile([128, 2, TQ], F32R, tag="es", name="est")
                for half in range(2):
                    nc.scalar.activation(
                        est[:, half, cs], _r(psd[:, half, cs]), EXPF, scale=0.125
                    )
                if dg >= 0:
                    # only the staircase band needs masking: cols >= c0+TK are
                    # fully valid (p + 128*dg <= c for all p<128), except dg=3
                    # where [c0, c0+TK) is fully masked and the staircase sits
                    # in [c0+TK, c0+2TK)
                    hi = c0 + (2 if dg == 3 else 1) * TK
                    ms = slice(c0, hi)
                    nc.vector.tensor_mul(
                        est[:, :, ms],
                        est[:, :, ms],
                        mk_sb[:, dg * TQ + c0 : dg * TQ + hi]
                        .unsqueeze(1)
                        .broadcast_to((128, 2, hi - c0)),
                    )
                for half in range(2):
                    h = pair * 2 + half
                    vap = vaug[:, j, h * (D + 1) : (h + 1) * (D + 1)]
                    nc.tensor.matmul(
                        AVD[:, half, cs],
                        vap,
                        est[:, half, cs],
                        start=(j == 0),
                        stop=(j == jmax - 1),
                    )
            # normalize: y = y_unnorm / denom (denom in row D of av psum)
            tqs = slice(i * TQ, (i + 1) * TQ)
            avs = rpool.tile([D + 1, 2, TQ], F32, tag="avs", name="avs")
            for half in range(2):
                nc.vector.tensor_copy(avs[:, half, :], AVD[:, half, :])
            # reciprocal lands in a f32r tile (row D, lane-aligned)
            r32 = rpool.tile([D + 1, 2, TQ], F32R, tag="r32", name="r32")
            with nc.allow_low_precision(reason="TF32 softmax denominators"):
                nc.vector.reciprocal(r32[D : D + 1, :, :], avs[D : D + 1, :, :])
            if last:
                # broadcast 1/denom with a rank-1 PE matmul into the AVD
                # banks: lowest latency, but holds AVD (fine on the last
                # block where no further attention needs it)
                ones_row = mk_sb[D : D + 1, 2 * TK : 2 * TK + D]
                for half in range(2):
                    nc.tensor.matmul(
                        AVD[0:D, half, :],
                        ones_row,
                        r32[D : D + 1, half, :],
                        start=True,
                        stop=True,
                    )
                rbA = AVD[0:D, 0, :]
                rbB = AVD[0:D, 1, :]
            else:
                # DRAM-bounce broadcast: higher latency but AVD stays free
                # so the next attention block's AV can start immediately
                scr = dpool.tile([2, TQ], F32R, tag="scr", name="scr")
                nc.sync.dma_start(out=scr[:], in_=r32[D : D + 1, :, :])
                rbt = [
                    rbpool.tile([D, TQ], F32R, tag=f"rb{h}", name=f"rb{h}")
                    for h in range(2)
                ]
                nc.sync.dma_start(
                    out=rbt[0][:], in_=scr[0:1, :].to_broadcast((D, TQ))
                )
                nc.sync.dma_start(
                    out=rbt[1][:], in_=scr[1:2, :].to_broadcast((D, TQ))
                )
                rbA = rbt[0][:]
                rbB = rbt[1][:]
            nc.vector.tensor_mul(
                yT[pair][0:D, tqs], _r(avs[0:D, 0, :]), rbA
            )
            nc.vector.tensor_mul(avs[0:D, 1, :], avs[0:D, 1, :], rbB)
            nc.sync.dma_start(
                out=yT[pair][D : 2 * D, tqs], in_=_r(avs[0:D, 1, :])
            )

        def proj_block(i):
            # on the last block there is no next qkv, so the PP bank is free:
            # ping-pong proj groups across PJ/PP to hide copy-waits in the tail
            last_blk = i == NB - 1
            for tt in range(TQ // TK):
                tq0 = i * TQ + tt * TK
                osb = opool.tile([128, C], F32, tag="o")
                for half in range(2):
                    po = (PJ + PP)[cnt["pj"] % 2] if last_blk else PJ[0]
                    cnt["pj"] += 1
                    for pair in range(2):
                        yap = yT[pair][:, tq0 : tq0 + TK]
                        wap = wp_sb[:, pair, half * 512 : (half + 1) * 512]
                        nc.tensor.matmul(
                            po[:],
                            yap,
                            wap,
                            start=(pair == 0),
                            stop=(pair == 1),
                        )
                    nc.vector.tensor_copy(
                        osb[:, half * 512 : (half + 1) * 512], _r(po[:])
                    )
                    nc.sync.dma_start(
                        out=out[tq0 : tq0 + TK, half * 512 : (half + 1) * 512],
                        in_=osb[:, half * 512 : (half + 1) * 512],
                    )

        for _rep in range(repeat):
            for tb in range(NB):
                qkv_block(tb, _rep)
                for pair in range(2):
                    attn_block(pair, tb, last=(tb == NB - 1))
                proj_block(tb)

    nc.compile()
    return nc


def _host_shards(x, w_qkv, w_proj):
    x = np.asarray(x, dtype=np.float32)
    w_qkv = np.asarray(w_qkv, dtype=np.float32)
    w_proj = np.asarray(w_proj, dtype=np.float32)

    p = np.arange(128)[:, None]
    c = np.arange(TQ)[None, :]
    masks = np.concatenate(
        [(p + d * TK <= c).astype(np.float32) for d in range(4)], axis=1
    )  # [128, 2048]

    in_maps = []
    for core in range(NCORES):
        b, g = divmod(core, NCORES // B)
        qc = w_qkv[:, g * GC : (g + 1) * GC]
        kc = w_qkv[:, C + g * GC : C + (g + 1) * GC]
        vc = w_qkv[:, 2 * C + g * GC : 2 * C + (g + 1) * GC]
        xt = x[b].T.reshape(CK, 128, NB, TQ).transpose(0, 2, 1, 3)
        wg = np.concatenate([qc, kc, vc], axis=1).reshape(CK, 128, 3 * GC)
        wp = w_proj[g * GC : (g + 1) * GC, :].reshape(2, 128, C)
        in_maps.append(
            {
                "xt": np.ascontiguousarray(xt),
                "wqkv": np.ascontiguousarray(wg),
                "wproj": np.ascontiguousarray(wp),
                "masks": masks,
            }
        )
    return in_maps


def kernel(x, w_qkv, w_proj, _trace=False, _trace_kwargs=None):
    if "nc" not in _CACHE:
        _CACHE["nc"] = _build_bass()
    nc = _CACHE["nc"]
    in_maps = _host_shards(x, w_qkv, w_proj)
    res = run_bass_kernel_spmd(
        nc,
        in_maps,
        core_ids=list(range(NCORES)),
        trace=_trace,
        **(_trace_kwargs or {}),
    )
    _CACHE["last_result"] = res
    g_per_b = NCORES // B
    out = np.stack(
        [
            np.sum([res.results[b * g_per_b + g]["out"] for g in range(g_per_b)], axis=0)
            for b in range(B)
        ]
    ).astype(np.float32)
    return out



# revision 33
# speedup vs baseline: 1.1357x; 1.1357x over previous
"""Causal self-attention (B=2, T=2048, C=1024, H=16) on 8 trn2 NeuronCores.

Sharding: core = (batch b, head-group g) with 4 heads per group.
  - data parallel over B (2 ways) x tensor parallel over heads (4 ways)
  - each core computes qkv for its head group, causal attention for its
    4 heads, and a partial proj (its 256 rows of w_proj); the host sums
    the 4 per-batch partials (deferred tensor-parallel all-reduce).

All SBUF/DRAM tensors are bf16 (PSUM accumulation fp32); rel-err gate is
2e-2 and bf16 end-to-end lands ~3e-3. bf16 keeps the PE at 1 cycle/row
at any moving size (no fp32r >=256 rule), so diagonal S/AV tiles compute
exactly the valid [dg*TK, TQ) range, and it halves DMA + DVE traffic.

Device layout (per core; host pre-arranges inputs so every DMA is a
large contiguous transfer — DMA issue costs a fixed ~625ns of shared
HWDGE time each, so few/large DMAs matter):
  xt    [4, 128, 8, 512]  x^T: (t-block, partition, c-chunk, t)
  wqkv  [128, 8, 768]     [q_g | k_g | v_g] chunks for this group
  wproj [128, 2, 1024]    w_proj row-chunks for this group
  tri   [128, 128]        causal triangle (tri[p,c] = c >= p)
  out   [2048, 1024]      bf16 partial output (host sums groups in f32)

No on-device transposes: q^T,k^T [d,t] come straight out of the qkv
matmul (lhsT = w slice, rhs = x^T); S^T[tk,tq] = k^T.T @ q^T-moving;
exp on ACT (both halves fused per tile); y^T plus the softmax
denominator come from one AV matmul with a ones-column appended to v;
proj consumes y^T directly as lhsT. Softmax skips max-subtraction
(scores ~ N(0,1) after 1/sqrt(D)). The denominator is broadcast across
partitions with a rank-1 PE matmul (ones column x denom row) into the
AV psum bank, then y = y_unnorm / denom on DVE — no reciprocal, no
DRAM-bounce broadcast DMAs.

Schedule: the S->exp->AV attention pipeline is ACT-bound per tile, so
qkv for block tb+1 (and proj for earlier blocks) are emitted as
"filler" matmul groups between attention tiles to keep the PE busy
during exp waits: qkv(tb+1) during attn(tb); proj(0) during attn(2);
proj(1), proj(2) during attn(3); proj(3) in the tail. PSUM->SBUF
copies run on the otherwise-idle Pool engine.
"""

import os
import sys
from contextlib import ExitStack

import numpy as np
import ml_dtypes

for _p in ("/opt/trn_rl_repo", "/root/.axon_site/_ro/trn_rl_repo"):
    if os.path.isdir(_p) and _p not in sys.path:
        sys.path.insert(0, _p)

import concourse.bass as bass
import concourse.bacc as bacc
import concourse.mybir as mybir
import concourse.tile as tile
from concourse.bass_utils import run_bass_kernel_spmd

B, T, C, H, D = 2, 2048, 1024, 16, 64
GH = 4                 # heads per core (group)
GC = GH * D            # 256 channels per group
NCORES = 8
TQ = 512               # query tile (free dim of S^T / AV matmuls)
TK = 128               # key tile (partition dim of S^T)
NB = T // TQ           # 4 query blocks
NT = T // TK           # 16 key tiles
CK = C // 128          # 8 contraction chunks for qkv
F32 = mybir.dt.float32
F32R = mybir.dt.float32r
BF16 = mybir.dt.bfloat16

EXPF = mybir.ActivationFunctionType.Exp
COPYF = mybir.ActivationFunctionType.Copy
DIV = mybir.AluOpType.divide

_CACHE = {}


def _r(ap):
    """View an fp32 AP as float32r (TF32; same 4-byte container)."""
    return ap.bitcast(mybir.dt.float32r)


def _build_bass(repeat=1):
    nc = bacc.Bacc("TRN2", target_bir_lowering=False, debug=False)
    xt = nc.declare_dram_parameter("xt", [NB, 128, CK, TQ], BF16, isOutput=False)
    wqkv = nc.declare_dram_parameter("wqkv", [128, CK, 3 * GC], BF16, isOutput=False)
    wproj = nc.declare_dram_parameter("wproj", [128, 2, C], BF16, isOutput=False)
    tri = nc.declare_dram_parameter("tri", [128, TK], BF16, isOutput=False)
    out = nc.declare_dram_parameter("out", [T, C], BF16, isOutput=True)

    with ExitStack() as ctx:
        tc = ctx.enter_context(tile.TileContext(nc))
        consts = ctx.enter_context(tc.tile_pool(name="consts", bufs=1))
        persist = ctx.enter_context(tc.tile_pool(name="persist", bufs=1))
        xpool = ctx.enter_context(tc.tile_pool(name="xp", bufs=2))
        espool = ctx.enter_context(tc.tile_pool(name="es", bufs=4))
        rpool = ctx.enter_context(tc.tile_pool(name="rp", bufs=2))
        rbpool = ctx.enter_context(tc.tile_pool(name="rb", bufs=2))
        opool = ctx.enter_context(tc.tile_pool(name="op", bufs=2))
        # 8 fixed PSUM tiles (one bank each), rotated manually. Pool-based
        # PSUM slot reuse emits unconditional multi-sem release waits on the
        # claiming matmul; with fixed tiles reuse is plain WAR/WAW dep
        # tracking.
        psum = ctx.enter_context(tc.tile_pool(name="psum", bufs=1, space="PSUM"))
        PSD = [
            psum.tile([128, 2, TQ], F32, tag=f"psd{r}", name=f"psd{r}")
            for r in range(2)
        ]
        PP = [psum.tile([128, TQ], F32, tag="pp0", name="pp0")]
        PJ = [psum.tile([128, TQ], F32, tag="pj0", name="pj0")]
        AVD = psum.tile([D + 1, 2, TQ], F32, tag="avd", name="avd")

        # ---- constants / persistent tiles ----
        w_sb = consts.tile([128, CK, 3 * GC], BF16, tag="wqkv")
        wp_sb = consts.tile([128, 2, C], BF16, tag="wproj")
        tri_sb = consts.tile([128, TK], BF16, tag="tri")
        ones_f32 = consts.tile([D + 1, D], F32R, tag="ones1")

        qT = [
            persist.tile([128, T], BF16, tag=f"qT{p}", name=f"qT{p}") for p in range(2)
        ]
        kT = [
            persist.tile([128, T], BF16, tag=f"kT{p}", name=f"kT{p}") for p in range(2)
        ]
        yT = [
            persist.tile([128, T], BF16, tag=f"yT{p}", name=f"yT{p}") for p in range(2)
        ]
        vaug = persist.tile([128, NT, GH * (D + 1)], BF16, tag="vaug")

        def dma_x(tb):
            x_sb = xpool.tile([128, CK, TQ], BF16, tag="x")
            nc.sync.dma_start(out=x_sb[:], in_=xt[tb])
            return x_sb

        def qkv_fillers(tb, x_sb, banks):
            """Yield emit-closures for block tb's qkv: 8 half-groups for
            q/k (4 outputs x 2 chunk-halves) + 4 v groups. `banks` is the
            psum tile rotation (1 = PP only, 2 = ping-pong)."""
            state = {"g": 0}

            def qk_half(pair, which, hlf):
                def emit():
                    pq = banks[state["g"] % len(banks)]
                    for k in range(4 * hlf, 4 * hlf + 4):
                        cols = which * GC + pair * 128
                        nc.tensor.matmul(
                            pq[:],
                            w_sb[:, k, cols : cols + 128],
                            x_sb[:, k, :],
                            start=(k == 0),
                            stop=(k == CK - 1),
                        )
                    if hlf == 1:
                        dest = qT if which == 0 else kT
                        nc.vector.tensor_copy(
                            dest[pair][:, tb * TQ : (tb + 1) * TQ], pq[:]
                        )
                        state["g"] += 1
                return emit

            def v_grp(tt):
                def emit():
                    jt = tb * (TQ // TK) + tt
                    pv = banks[state["g"] % len(banks)]
                    state["g"] += 1
                    for k in range(CK):
                        nc.tensor.matmul(
                            pv[:, 0:GC],
                            x_sb[:, k, tt * TK : (tt + 1) * TK],
                            w_sb[:, k, 2 * GC : 3 * GC],
                            start=(k == 0),
                            stop=(k == CK - 1),
                        )
                    nc.vector.tensor_copy(
                        vaug[:, jt, :]
                        .rearrange("p (h c) -> p h c", h=GH)[:, :, 0:D],
                        pv[:, 0:GC].rearrange("p (h c) -> p h c", h=GH),
                    )
                return emit

            for pair in range(2):
                for which in range(2):
                    yield qk_half(pair, which, 0)
                    yield qk_half(pair, which, 1)
            for tt in range(TQ // TK):
                yield v_grp(tt)

        def proj_fillers(i, banks, tail=False):
            """Yield emit-closures for block i's proj: 8 groups of
            (tt, half), each 2 accumulating matmuls + psum->sbuf copy.
            Output DMAs are per-tt [TK, C]; on the tail block they are
            per-half and the copies alternate DVE/Pool so the copy
            stream keeps up with the PE."""
            state = {"g": 0, "osb": None}

            def grp(tt, half):
                def emit():
                    tq0 = i * TQ + tt * TK
                    if state["osb"] is None:
                        state["osb"] = opool.tile(
                            [128, TQ // TK, C], BF16, tag="o", name="osb"
                        )
                    osb = state["osb"]
                    po = banks[state["g"] % len(banks)]
                    state["g"] += 1
                    for pair in range(2):
                        nc.tensor.matmul(
                            po[:],
                            yT[pair][:, tq0 : tq0 + TK],
                            wp_sb[:, pair, half * 512 : (half + 1) * 512],
                            start=(pair == 0),
                            stop=(pair == 1),
                        )
                    if tail:
                        nc.scalar.activation(
                            osb[:, tt, half * 512 : (half + 1) * 512],
                            po[:],
                            COPYF,
                        )
                    else:
                        nc.vector.tensor_copy(
                            osb[:, tt, half * 512 : (half + 1) * 512], po[:]
                        )
                    if half == 1:
                        nc.sync.dma_start(
                            out=out[tq0 : tq0 + TK, :], in_=osb[:, tt, :]
                        )
                return emit

            for tt in range(TQ // TK):
                for half in range(2):
                    yield grp(tt, half)

        def attn_block(pair, i, fillers, fstate=None):
            """Software-pipelined S->exp->AV over key tiles j, with
            fillers spread evenly across the block's AV slots."""
            jmax = (TQ // TK) * (i + 1)
            pend = None  # (j, est, cs)
            if fstate is None:
                fstate = {"slots": jmax, "acc": 0.0}

            def maybe_fill():
                if fillers:
                    fillers.pop(0)()

            def av(j, est, cs):
                for half in range(2):
                    h = pair * 2 + half
                    vap = vaug[:, j, h * (D + 1) : (h + 1) * (D + 1)]
                    nc.tensor.matmul(
                        AVD[:, half, cs],
                        vap,
                        est[:, half, cs],
                        start=(j == 0),
                        stop=(j == jmax - 1),
                    )

            for j in range(jmax):
                dg = j - (TQ // TK) * i  # >=0 on the diagonal band
                c0 = dg * TK if dg > 0 else 0
                cs = slice(c0, TQ)
                psd = PSD[j % 2]
                for half in range(2):
                    lo, hi = half * D, half * D + D
                    kap = kT[pair][lo:hi, j * TK : (j + 1) * TK]
                    qap = qT[pair][lo:hi, i * TQ + c0 : (i + 1) * TQ]
                    nc.tensor.matmul(
                        psd[:, half, cs], kap, qap, start=True, stop=True
                    )
                est = espool.tile([128, 2, TQ], BF16, tag="es", name="est")
                nc.scalar.activation(
                    est[:, :, cs], psd[:, :, cs], EXPF, scale=0.125
                )
                if dg >= 0:
                    # the staircase band [c0, c0+TK) is the only partially
                    # valid region; cols >= c0+TK are fully valid
                    ms = slice(c0, c0 + TK)
                    nc.gpsimd.tensor_mul(
                        est[:, :, ms],
                        est[:, :, ms],
                        tri_sb[:, :].unsqueeze(1).broadcast_to((128, 2, TK)),
                    )
                if pend is not None:
                    av(*pend)
                    maybe_fill()
                pend = (j, est, cs)
            av(*pend)
            maybe_fill()

            # normalize: y = y_unnorm / denom (denom in row D of av psum).
            # Copy the unnormalized AV out, rank-1-broadcast the denom row
            # into the AV psum rows, then divide on DVE. Half B first so
            # its SBUF->SBUF partition-move DMA overlaps half A's divide.
            tqs = slice(i * TQ, (i + 1) * TQ)
            avs = rpool.tile([D + 1, 2, TQ], F32, tag="avs", name="avs")
            for half in range(2):
                nc.vector.tensor_copy(avs[:, half, :], AVD[:, half, :])
            r32 = rpool.tile([D + 1, 2, TQ], F32R, tag="r32", name="r32")
            with nc.allow_low_precision(reason="TF32 softmax denominators"):
                nc.vector.reciprocal(r32[D : D + 1, :, :], avs[D : D + 1, :, :])
            for half in (1, 0):
                nc.tensor.matmul(
                    AVD[0:D, half, :],
                    ones_f32[D : D + 1, :],
                    r32[D : D + 1, half, :],
                    start=True,
                    stop=True,
                )
            yB = rbpool.tile([D, TQ], BF16, tag="yB", name="yB")
            nc.vector.tensor_mul(yB[:], _r(avs[0:D, 1, :]), _r(AVD[0:D, 1, :]))
            nc.sync.dma_start(out=yT[pair][D : 2 * D, tqs], in_=yB[:])
            nc.vector.tensor_mul(
                yT[pair][0:D, tqs], _r(avs[0:D, 0, :]), _r(AVD[0:D, 0, :])
            )

        for _rep in range(repeat):
            # startup: consts + x(0) + weights, ordered so the first qkv
            # matmuls start as early as possible
            nc.sync.dma_start(out=tri_sb[:], in_=tri[:])
            nc.vector.tensor_copy(ones_f32[D : D + 1, :], tri_sb[D : D + 1, TK - D : TK])
            for h in range(GH):
                nc.gpsimd.tensor_copy(
                    vaug[:, :, h * (D + 1) + D : (h + 1) * (D + 1)],
                    tri_sb[:, TK - 1 : TK].unsqueeze(1).broadcast_to((128, NT, 1)),
                )
            x_cur = xpool.tile([128, CK, TQ], BF16, tag="x")
            nc.sync.dma_start(
                out=w_sb[:, 0:4, 0 : 2 * GC], in_=wqkv[:, 0:4, 0 : 2 * GC]
            )
            for kk in range(2):
                nc.sync.dma_start(
                    out=x_cur[:, 2 * kk : 2 * kk + 2, :],
                    in_=xt[0, :, 2 * kk : 2 * kk + 2, :],
                )
            nc.sync.dma_start(
                out=w_sb[:, 4:8, 0 : 2 * GC], in_=wqkv[:, 4:8, 0 : 2 * GC]
            )
            for kk in range(2, 4):
                nc.sync.dma_start(
                    out=x_cur[:, 2 * kk : 2 * kk + 2, :],
                    in_=xt[0, :, 2 * kk : 2 * kk + 2, :],
                )
            nc.sync.dma_start(
                out=w_sb[:, :, 2 * GC : 3 * GC], in_=wqkv[:, :, 2 * GC : 3 * GC]
            )
            # qkv(0) runs un-interleaved (nothing to overlap with yet);
            # ping-pong PP/PJ since proj is idle until attn(2)
            for f in qkv_fillers(0, x_cur, PP + PJ):
                f()
            nc.sync.dma_start(out=wp_sb[:], in_=wproj[:])

            for tb in range(NB):
                fillers = []
                if tb + 1 < NB:
                    x_nxt = dma_x(tb + 1)
                    banks = (PP + PJ) if tb == 0 else PP
                    fillers += list(qkv_fillers(tb + 1, x_nxt, banks))
                if tb == 2:
                    fillers += list(proj_fillers(0, PJ))
                elif tb == 3:
                    fillers += list(proj_fillers(1, PJ))
                    fillers += list(proj_fillers(2, PJ))
                fstate = {"slots": 8 * (tb + 1)}
                for pair in range(2):
                    attn_block(pair, tb, fillers, fstate)
                while fillers:
                    fillers.pop(0)()
            for f in proj_fillers(NB - 1, PJ + PP, tail=True):
                f()

    nc.compile()
    return nc


def _host_shards(x, w_qkv, w_proj):
    x = np.asarray(x, dtype=np.float32)
    w_qkv = np.asarray(w_qkv, dtype=np.float32)
    w_proj = np.asarray(w_proj, dtype=np.float32)
    bf = ml_dtypes.bfloat16

    p = np.arange(128)[:, None]
    c = np.arange(TK)[None, :]
    tri = (p <= c).astype(bf)  # [128, 128]

    in_maps = []
    for core in range(NCORES):
        b, g = divmod(core, NCORES // B)
        qc = w_qkv[:, g * GC : (g + 1) * GC]
        kc = w_qkv[:, C + g * GC : C + (g + 1) * GC]
        vc = w_qkv[:, 2 * C + g * GC : 2 * C + (g + 1) * GC]
        # xt[tb, p, k, t] = x[b][tb*TQ + t, 128*k + p]
        xt = x[b].reshape(NB, TQ, CK, 128).transpose(0, 3, 2, 1)
        # wqkv[p, k, c] = w_group[128*k + p, c]
        wg = np.concatenate([qc, kc, vc], axis=1).reshape(CK, 128, 3 * GC)
        wg = wg.transpose(1, 0, 2)
        # wproj[p, kk, c] = w_proj[g*GC + 128*kk + p, c]
        wp = w_proj[g * GC : (g + 1) * GC, :].reshape(2, 128, C).transpose(1, 0, 2)
        in_maps.append(
            {
                "xt": np.ascontiguousarray(xt).astype(bf),
                "wqkv": np.ascontiguousarray(wg).astype(bf),
                "wproj": np.ascontiguousarray(wp).astype(bf),
                "tri": tri,
            }
        )
    return in_maps


def kernel(x, w_qkv, w_proj, _trace=False, _trace_kwargs=None):
    if "nc" not in _CACHE:
        _CACHE["nc"] = _build_bass()
    nc = _CACHE["nc"]
    in_maps = _host_shards(x, w_qkv, w_proj)
    res = run_bass_kernel_spmd(
        nc,
        in_maps,
        core_ids=list(range(NCORES)),
        trace=_trace,
        **(_trace_kwargs or {}),
    )
    _CACHE["last_result"] = res
    g_per_b = NCORES // B
    out = np.stack(
        [
            np.sum(
                [
                    np.asarray(res.results[b * g_per_b + g]["out"], dtype=np.float32)
                    for g in range(g_per_b)
                ],
                axis=0,
            )
            for b in range(B)
        ]
    ).astype(np.float32)
    return out
